# revision 26
# baseline (speedup 1.0000x reference)
"""MoE layer (router + 8 experts top-2 + shared expert) on 8 Trainium2 cores.

Strategy (expert-parallel, matching the all-to-all dispatch hint):
  - Host computes router logits/top-2/softmax and gathers each expert's
    tokens (the "all-to-all dispatch" — host-side since kernel() owns the
    full inputs and sharding).
  - Core c holds expert c's W1/W2 fully resident in SBUF (bf16, 128 KiB of
    the ~208 KiB/partition) and computes
        y_c = relu(x_gathered @ W1_c + b1_c) @ W2_c
    for its (padded) token set in <=512-token chunks: layer 1 materializes
    the full d_ff hidden state for the chunk in SBUF, then layer 2
    accumulates all 32 d_ff tiles directly in PSUM (no SBUF re-accumulate).
  - The always-on shared expert is d_ff-sliced 8 ways: core c computes
    partial_c = relu(x_all @ Ws1[:, c*512:(c+1)*512] + bs1[slice]) @ Ws2[slice]
    over all tokens in ~344-token chunks; partials are summed on host.
  - Host applies gate weights, b2/bs2 biases, and scatter-adds expert
    outputs back to token order.

All matmul operands are bf16 (PSUM accumulation stays fp32): that halves
HBM traffic and SBUF footprint vs fp32 while keeping the full PE rate, and
the end-to-end quantization error (~3e-3 rel) is far inside the 2e-2 gate.
Chunk widths stay in the ~344-368 moving-dim range: HW-measured matmul
streams run ~2x faster per column there than at the 512 PSUM-bank limit.
"""

import os
import sys

import ml_dtypes
import numpy as np

for _p in ("/opt/trn_rl_repo", os.path.expanduser("~/.axon_site/_ro/trn_rl_repo")):
    if os.path.isdir(_p) and _p not in sys.path:
        sys.path.append(_p)

import concourse.bass as bass  # noqa: E402
import concourse.tile as tile  # noqa: E402
from concourse import bacc, mybir  # noqa: E402
from concourse.bass import ds, ts  # noqa: E402
from concourse.bass_utils import run_bass_kernel_spmd  # noqa: E402

D_MODEL, D_FF, N_EXP, TOP_K = 1024, 4096, 8, 2
P = 128
KD = D_MODEL // P        # 8 partition-tiles over d_model
MF = D_FF // P           # 32 partition-tiles over d_ff
FF_SH = D_FF // N_EXP    # 512: shared-expert d_ff slice per core
MS = FF_SH // P          # 4 partition-tiles over the shared slice
T_TOTAL = 4096
TT = 344                 # shared-expert token chunk (<=~380: fast PE zone)
MPACK = 4                # w1/w2 DMA granularity: 4 d_ff tiles per pack

F32 = mybir.dt.float32
BF16 = mybir.dt.bfloat16
NP_BF16 = ml_dtypes.bfloat16
RELU = mybir.ActivationFunctionType.Relu
ADD = mybir.AluOpType.add
MAX = mybir.AluOpType.max


def _evac_relu(nc, use_vector, out, ps, bias):
    """relu(psum + bias) -> out, alternating between scalar and vector
    engines so neither becomes the evacuation bottleneck."""
    if use_vector:
        nc.vector.tensor_scalar(
            out=out, in0=ps, scalar1=bias, scalar2=0.0, op0=ADD, op1=MAX
        )
    else:
        nc.scalar.activation(out=out, in_=ps, func=RELU, bias=bias)


def _evac_copy(nc, use_vector, out, ps):
    if use_vector:
        nc.vector.tensor_scalar(
            out=out, in0=ps, scalar1=0.0, scalar2=None, op0=ADD
        )
    else:
        nc.scalar.copy(out=out, in_=ps)


def _chunks(n, step=512):
    """Near-equal chunks of at most `step` columns, widths multiple of 4.
    Keeping every chunk >=256 keeps the PE streaming efficiency high and
    every chunk fits one PSUM bank (<=512 fp32)."""
    if n <= step:
        return [(0, n)]
    cnt = -(-n // step)
    w = -(-n // (cnt * 4)) * 4
    out, off = [], 0
    while off + w < n:
        out.append((off, w))
        off += w
    out.append((off, n - off))
    return out


def _declare_io(nc, C, external_inputs=True):
    if external_inputs:
        def inp(name, shape, dt):
            return nc.declare_dram_parameter(name, shape, dt, isOutput=False)
    else:
        def inp(name, shape, dt):
            return nc.dram_tensor(name, shape, dt)
    t = {}
    t["xg"] = inp("xg", [P, KD, C], BF16)
    t["xt"] = inp("xt", [P, KD, T_TOTAL], BF16)
    t["w1"] = inp("w1", [P, MF, KD, P], BF16)
    t["w2"] = inp("w2", [P, MF, D_MODEL], BF16)
    t["b1t"] = inp("b1t", [P, MF], F32)
    t["ws1"] = inp("ws1", [P, KD, FF_SH], BF16)
    t["ws2"] = inp("ws2", [P, MS, D_MODEL], BF16)
    t["bs1t"] = inp("bs1t", [P, MS], F32)
    t["yt"] = nc.declare_dram_parameter("yt", [P, KD, C], BF16, isOutput=True)
    t["st"] = nc.declare_dram_parameter("st", [P, KD, T_TOTAL], BF16, isOutput=True)
    return t


def _emit_body(nc, tc, t, C):
    chunks = _chunks(C)
    with (
        tc.tile_pool(name="const1", bufs=1) as const1,
        tc.tile_pool(name="ph", bufs=3, space="PSUM") as ph,
        tc.tile_pool(name="py", bufs=3, space="PSUM") as py,
    ):
        b1_sb = const1.tile([P, MF], F32, tag="b1")
        bs1_sb = const1.tile([P, MS], F32, tag="bs1")
        ws1_sb = const1.tile([P, KD, FF_SH], BF16, tag="ws1")
        ws2_sb = const1.tile([P, MS, D_MODEL], BF16, tag="ws2")
        _phase1(nc, tc, t, C, chunks, b1_sb, bs1_sb, ws1_sb, ws2_sb, ph, py)
        _phase2(nc, tc, t, bs1_sb, ws1_sb, ws2_sb, ph, py)


def _phase1(nc, tc, t, C, chunks, b1_sb, bs1_sb, ws1_sb, ws2_sb, ph, py):
    """This core's expert on its gathered tokens."""
    sync = nc.sync
    xg, w1, w2, b1t, ws1, ws2, bs1t, yt = (
        t["xg"], t["w1"], t["w2"], t["b1t"],
        t["ws1"], t["ws2"], t["bs1t"], t["yt"],
    )
    with (
        tc.tile_pool(name="w1_p", bufs=1) as w1_p,
        tc.tile_pool(name="w2_p", bufs=1) as w2_p,
        tc.tile_pool(name="xg_p", bufs=1) as xg_p,
        tc.tile_pool(name="h_p", bufs=1) as h_p,
        tc.tile_pool(name="y_p", bufs=1) as y_p,
    ):
        xg_sb = xg_p.tile([P, KD, C], BF16, tag="xg")
        # w1/w2 as per-pack tiles so each matmul depends only on its own
        # pack's DMA (explicit fine-grained pipelining of the cold start).
        w1_sb = [
            w1_p.tile([P, MPACK, KD, P], BF16, tag=f"w1_{mp}", name=f"w1_{mp}")
            for mp in range(MF // MPACK)
        ]
        w2_sb = [
            w2_p.tile([P, MPACK, D_MODEL], BF16, tag=f"w2_{mp}", name=f"w2_{mp}")
            for mp in range(MF // MPACK)
        ]
        h_sb = h_p.tile([P, MF, 512], BF16, tag="h")

        # Startup DMAs in consumption order (single HWDGE ring is FIFO).
        sync.dma_start(out=b1_sb[:], in_=b1t[:])
        sync.dma_start(out=xg_sb[:], in_=xg[:])
        for mp in range(MF // MPACK):
            sync.dma_start(out=w1_sb[mp][:], in_=w1[:, ds(mp * MPACK, MPACK)])
        for mp in range(MF // MPACK):
            sync.dma_start(out=w2_sb[mp][:], in_=w2[:, ds(mp * MPACK, MPACK)])
        sync.dma_start(out=ws1_sb[:], in_=ws1[:])
        sync.dma_start(out=ws2_sb[:], in_=ws2[:])
        sync.dma_start(out=bs1_sb[:], in_=bs1t[:])

        for off, ln in chunks:
            # ---- layer 1: h = relu(x @ W1 + b1), full d_ff in SBUF
            for m in range(MF):
                ps = ph.tile([P, ln], F32, tag="ph")
                lw = w1_sb[m // MPACK]
                for k in range(KD):
                    nc.tensor.matmul(
                        ps[:],
                        lw[:, m % MPACK, k, :],
                        xg_sb[:, k, ds(off, ln)],
                        start=(k == 0),
                        stop=(k == KD - 1),
                    )
                _evac_relu(
                    nc, m % 2, h_sb[:, m, ds(0, ln)], ps[:], b1_sb[:, m : m + 1]
                )
            # ---- layer 2: y = h @ W2, all 32 d_ff tiles accumulated in PSUM
            y_sb = y_p.tile([P, KD, 512], BF16, tag="y")
            for j in range(KD):
                ps = py.tile([P, ln], F32, tag="py")
                for m in range(MF):
                    nc.tensor.matmul(
                        ps[:],
                        w2_sb[m // MPACK][:, m % MPACK, ts(j, P)],
                        h_sb[:, m, ds(0, ln)],
                        start=(m == 0),
                        stop=(m == MF - 1),
                    )
                _evac_copy(nc, j % 2, y_sb[:, j, ds(0, ln)], ps[:])
            sync.dma_start(out=yt[:, :, ds(off, ln)], in_=y_sb[:, :, ds(0, ln)])


def _phase2(nc, tc, t, bs1_sb, ws1_sb, ws2_sb, ph, py):
    """Shared expert, this core's d_ff slice, all tokens."""
    sync = nc.sync
    xt, st = t["xt"], t["st"]
    with (
        tc.tile_pool(name="xt_p", bufs=1) as xt_p,
        tc.tile_pool(name="hs_p", bufs=2) as hs_p,
        tc.tile_pool(name="st_p", bufs=2) as st_p,
    ):
        # all tokens resident (64 KiB/partition; phase-1 pools are freed)
        xt_sb = xt_p.tile([P, KD, T_TOTAL], BF16, tag="xt")
        for k in range(KD):
            sync.dma_start(out=xt_sb[:, k, :], in_=xt[:, k, :])
        for off, ln in _chunks(T_TOTAL, step=TT):
            hs_sb = hs_p.tile([P, MS, TT], BF16, tag="hs")
            for m in range(MS):
                ps = ph.tile([P, ln], F32, tag="ph")
                for k in range(KD):
                    nc.tensor.matmul(
                        ps[:],
                        ws1_sb[:, k, ts(m, P)],
                        xt_sb[:, k, ds(off, ln)],
                        start=(k == 0),
                        stop=(k == KD - 1),
                    )
                _evac_relu(
                    nc, m % 2, hs_sb[:, m, ds(0, ln)], ps[:], bs1_sb[:, m : m + 1]
                )
            st_sb = st_p.tile([P, KD, TT], BF16, tag="st")
            for j in range(KD):
                ps = py.tile([P, ln], F32, tag="py")
                for m in range(MS):
                    nc.tensor.matmul(
                        ps[:],
                        ws2_sb[:, m, ts(j, P)],
                        hs_sb[:, m, ds(0, ln)],
                        start=(m == 0),
                        stop=(m == MS - 1),
                    )
                _evac_copy(nc, j % 2, st_sb[:, j, ds(0, ln)], ps[:])
            sync.dma_start(out=st[:, :, ds(off, ln)], in_=st_sb[:, :, ds(0, ln)])


def build_program(C):
    nc = bacc.Bacc(None, target_bir_lowering=False, debug=False)
    t = _declare_io(nc, C, external_inputs=True)
    with tile.TileContext(nc) as tc:
        _emit_body(nc, tc, t, C)
    nc.compile()
    return nc


def build_timing_program(C, trip):
    """Timing variant: inputs are Internal DRAM (no host transfer), body
    repeated `trip` times in a hardware loop."""
    nc = bacc.Bacc(None, target_bir_lowering=False, debug=False)
    t = _declare_io(nc, C, external_inputs=False)
    with tile.TileContext(nc) as tc:
        with tc.For_i(0, trip, 1):
            _emit_body(nc, tc, t, C)
    nc.compile()
    return nc


def _to_tiles(a2d, dt=NP_BF16):
    """[R, N] with R = r_tiles*128 -> [128, r_tiles, N] so element
    [p, r, n] = a2d[r*128 + p, n]; contiguous for a single straight DMA."""
    R, N = a2d.shape
    return np.ascontiguousarray(
        a2d.reshape(R // P, P, N).transpose(1, 0, 2).astype(dt)
    )


def _from_tiles(a3d):
    """Inverse of _to_tiles: [128, r_tiles, N] -> [r_tiles*128, N]."""
    p, r, n = a3d.shape
    return a3d.astype(np.float32).transpose(1, 0, 2).reshape(r * p, n)


def _route(xf, Wg):
    """Replicates TopKRouter eval: top-2 by logit, softmax over the two."""
    logits = xf @ Wg
    top_idx = np.argsort(-logits, axis=1, kind="stable")[:, :TOP_K]
    top_vals = np.take_along_axis(logits, top_idx, axis=1)
    e = np.exp(top_vals - top_vals.max(axis=1, keepdims=True))
    top_w = (e / e.sum(axis=1, keepdims=True)).astype(np.float32)
    return top_idx, top_w


_PROG_CACHE = {}


def _get_program(C):
    if C not in _PROG_CACHE:
        _PROG_CACHE[C] = build_program(C)
    return _PROG_CACHE[C]


def make_in_maps(x, Wg, W1, b1, W2, b2, Ws1, bs1, Ws2, bs2):
    """Host-side routing + sharding. Returns (in_maps, C, idx_e, gate_e, xf)."""
    B, S, D = x.shape
    T = B * S
    xf = np.ascontiguousarray(np.asarray(x, np.float32).reshape(T, D))
    top_idx, top_w = _route(xf, np.asarray(Wg, np.float32))

    idx_e, gate_e = [], []
    for ex in range(N_EXP):
        rows, slot = np.nonzero(top_idx == ex)
        idx_e.append(rows)
        gate_e.append(top_w[rows, slot])
    counts = [len(i) for i in idx_e]
    C = max(8, -(-max(counts) // 8) * 8)

    # all-token activations: xt[p, k, t] = xf[t, k*128+p]
    xt_t = _to_tiles(np.ascontiguousarray(xf.T))
    in_maps = []
    for ex in range(N_EXP):
        xg = np.zeros((C, D_MODEL), np.float32)
        xg[: counts[ex]] = xf[idx_e[ex]]
        sl = slice(ex * FF_SH, (ex + 1) * FF_SH)
        # w1[p, m, k, j] = W1[ex][k*128+p, m*128+j]  (m-major packs)
        w1_t = np.ascontiguousarray(
            np.asarray(W1[ex], np.float32)
            .reshape(KD, P, MF, P)
            .transpose(1, 2, 0, 3)
            .astype(NP_BF16)
        )
        in_maps.append(
            {
                "xg": _to_tiles(np.ascontiguousarray(xg.T)),
                "xt": xt_t,
                "w1": w1_t,
                "w2": _to_tiles(np.asarray(W2[ex], np.float32)),
                "b1t": np.ascontiguousarray(
                    np.asarray(b1[ex], np.float32).reshape(MF, P).T
                ),
                "ws1": _to_tiles(np.asarray(Ws1[:, sl], np.float32)),
                "ws2": _to_tiles(np.asarray(Ws2[sl, :], np.float32)),
                "bs1t": np.ascontiguousarray(
                    np.asarray(bs1[sl], np.float32).reshape(MS, P).T
                ),
            }
        )
    return in_maps, C, idx_e, gate_e, xf


def assemble_output(results, shape, C, idx_e, gate_e, b2, bs2):
    B, S, D = shape
    T = B * S
    out = np.zeros((T, D), np.float32)
    for ex in range(N_EXP):
        out += _from_tiles(results[ex]["st"]).T  # shared partials
    out += np.asarray(bs2, np.float32)[None, :]
    b2 = np.asarray(b2, np.float32)
    for ex in range(N_EXP):
        y = _from_tiles(results[ex]["yt"]).T[: len(idx_e[ex])]
        out[idx_e[ex]] += gate_e[ex][:, None] * (y + b2[ex][None, :])
    return out.reshape(B, S, D)


def kernel(x, Wg, W1, b1, W2, b2, Ws1, bs1, Ws2, bs2):
    in_maps, C, idx_e, gate_e, _ = make_in_maps(
        x, Wg, W1, b1, W2, b2, Ws1, bs1, Ws2, bs2
    )
    nc = _get_program(C)
    res = run_bass_kernel_spmd(nc, in_maps, list(range(N_EXP)))
    return assemble_output(
        res.results, x.shape, C, idx_e, gate_e, b2, bs2
    ).astype(np.float32)
